# revision 12
# baseline (speedup 1.0000x reference)
"""8-layer LSTM (ProteinGenerator) on 8 NeuronCores: layer-per-core ring pipeline.

Core l runs layer l. Time is split into C chunks of S steps. Each wavefront
tick, every core AllGathers the hidden-state chunk it finished last tick
(feature-major, pre-transposed), and consumes the chunk its upstream neighbor
contributed one tick earlier (2-tick stage lag). Layer 0's input is a
host-precomputed one-hot token stream; the embedding+input matmul is folded
into a [23+pad, 1024] table so core 0 runs the identical program. A per-tick
bias stream sets the i/f gate biases to -30 before a core's start tick, which
flushes h/c to ~0 so no explicit state reset / control flow is needed.
"""

import sys

sys.path.insert(0, "/opt/trn_rl_repo")

import numpy as np

import concourse.bass as bass
import concourse.mybir as mybir
import concourse.tile as tile
from concourse.bass_utils import run_bass_kernel_spmd

F32 = mybir.dt.float32
AF = mybir.ActivationFunctionType

B, T, H, G, V, L = 64, 512, 256, 1024, 23, 8
NCORES = 8
S = 8                 # steps per chunk
C = T // S            # chunks
LAG = 2               # wavefront stage lag in ticks (compute + AG in flight)
TICKS = C + LAG * (L - 1)
KT = 2                # K tiles of 128 over H=256
FLUSH = -30.0

_cache = {}


def _build_nc():
    nc = bass.Bass(num_devices=NCORES, detect_race_conditions=False)

    wih = nc.dram_tensor("wih", [KT, 128, G], F32, kind="ExternalInput")
    whh = nc.dram_tensor("whh", [KT, 128, G], F32, kind="ExternalInput")
    fcw = nc.dram_tensor("fcw", [KT, 128, V], F32, kind="ExternalInput")
    biasstream = nc.dram_tensor("biasstream", [TICKS, G], F32, kind="ExternalInput")
    oneh = nc.dram_tensor("oneh", [C, 128, S * B], F32, kind="ExternalInput")
    ident = nc.dram_tensor("ident", [64, 64], F32, kind="ExternalInput")
    gidx = nc.dram_tensor("gidx", [128, 1], mybir.dt.int32, kind="ExternalInput")
    y_out = nc.dram_tensor("y", [TICKS, S, B, V], F32, kind="ExternalOutput")

    with tile.TileContext(nc) as tc:
        with (
            tc.tile_pool(name="const", bufs=1) as cpool,
            tc.tile_pool(name="state", bufs=1) as spool,
            tc.tile_pool(name="work", bufs=2) as wpool,
            tc.tile_pool(name="psg", bufs=2, space="PSUM") as psg,
            tc.tile_pool(name="pst", bufs=2, space="PSUM") as pst,
            tc.tile_pool(name="psy", bufs=2, space="PSUM") as psy,
            tc.tile_pool(name="dram", bufs=1, space="DRAM") as dpool,
        ):
            # ---- persistent SBUF ----
            wih_sb = cpool.tile([128, KT, G], F32, tag="wih")
            whh_sb = cpool.tile([128, KT, G], F32, tag="whh")
            fc_sb = cpool.tile([128, KT, V], F32, tag="fc")
            id_sb = cpool.tile([64, 64], F32, tag="id")
            ones_sb = cpool.tile([1, 128], F32, tag="ones")
            zero_sb = cpool.tile([128, KT * S * B], F32, tag="zero")
            gidx_sb = cpool.tile([128, 1], mybir.dt.int32, tag="gidx")

            s_sb = spool.tile([64, 1024], F32, tag="s")      # [i f | g | c]
            o_sb = spool.tile([64, H], F32, tag="o")
            tc_sb = spool.tile([64, H], F32, tag="tc")
            h_sb = spool.tile([64, H], F32, tag="h")
            p_sb = spool.tile([64, 512], F32, tag="p")
            # hT chunk buffers (double buffered across ticks), feature-major
            hTc = [spool.tile([128, KT, S, B], F32, name=f"hTc{i}", tag=f"hTc{i}") for i in range(2)]

            for k in range(KT):
                nc.sync.dma_start(wih_sb[:, k, :], wih[k])
                nc.sync.dma_start(whh_sb[:, k, :], whh[k])
                nc.sync.dma_start(fc_sb[:, k, :], fcw[k])
            nc.sync.dma_start(id_sb[:], ident[:])
            nc.sync.dma_start(gidx_sb[:], gidx[:])
            nc.gpsimd.memset(ones_sb[:], 1.0)
            nc.gpsimd.memset(zero_sb[:], 0.0)
            nc.gpsimd.memset(s_sb[:], 0.0)
            for i in range(2):
                nc.gpsimd.memset(hTc[i][:], 0.0)

            # ---- DRAM comm buffers: [9 slots, KT, 128, S*B]; slot 0 = onehot
            # scratch, slots 1..8 = AllGather output of ranks 0..7. Two tensors
            # (parity ping-pong) so tick-w reads never alias tick-w AG writes.
            ag_out = [dpool.tile([9, 128, KT * S * B], F32, name=f"agout{i}", tag=f"agout{i}") for i in range(2)]
            ag_in = [dpool.tile([128, KT * S * B], F32, name=f"agin{i}", tag=f"agin{i}") for i in range(2)]

            _z = zero_sb[:]
            _zb = bass.AP(
                _z.tensor, _z.offset,
                [[KT * S * B, 128], [0, 9], [1, KT * S * B]],
            )
            for i in range(2):
                nc.sync.dma_start(
                    ag_out[i][:].rearrange("s p f -> p s f"), _zb
                )
                nc.sync.dma_start(
                    ag_in[i][:], zero_sb[:]
                )

            for w in range(TICKS):
                par = w % 2
                rpar = (w - 1) % 2
                prev_hT = hTc[(w - 1) % 2]
                cur_hT = hTc[w % 2]

                # contribution: last tick's hT chunk -> bounce -> AllGather
                nc.sync.dma_start(
                    ag_in[par][:], prev_hT[:].rearrange("p k s b -> p (k s b)")
                )
                nc.gpsimd.collective_compute(
                    "AllGather",
                    mybir.AluOpType.bypass,
                    replica_groups=[list(range(NCORES))],
                    ins=[ag_in[par][:].opt()],
                    outs=[ag_out[par][1:9].opt()],
                )

                # onehot chunk for this tick into read-parity slot 0 (used by core 0)
                nc.sync.dma_start(ag_out[rpar][0, :, 0 : S * B], oneh[w % C])

                # load input chunk: slot index == pid (slot0 = onehot for pid 0,
                # slot 1+r = rank r's h for pid = r+1)
                hin = wpool.tile([128, KT, S, B], F32, tag="hin")
                nc.gpsimd.indirect_dma_start(
                    out=hin[:].rearrange("p k s b -> p (k s b)"),
                    out_offset=None,
                    in_=ag_out[rpar][:].rearrange("s p f -> (s p) f"),
                    in_offset=bass.IndirectOffsetOnAxis(ap=gidx_sb[:, :1], axis=0),
                )

                bias_sb = wpool.tile([1, G], F32, tag="bias")
                nc.sync.dma_start(bias_sb[:], biasstream[w : w + 1, :])

                y_sb = wpool.tile([128, S // 2, V], F32, tag="ysb")

                gA = gB = None
                for t in range(S):
                    r = t % 2
                    if r == 0:
                        gA = psg.tile([128, 512], F32, tag="gA")
                        gB = psg.tile([128, 512], F32, tag="gB")
                        # input GEMM for steps t, t+1 (+ gate bias)
                        for k in range(KT):
                            lhs = hin[:, k, t : t + 2, :].rearrange("p s b -> p (s b)")
                            nc.tensor.matmul(
                                gA[:], lhs, wih_sb[:, k, 0:512],
                                start=(k == 0), stop=False,
                            )
                            nc.tensor.matmul(
                                gB[:], lhs, wih_sb[:, k, 512:1024],
                                start=(k == 0), stop=False,
                            )
                        nc.tensor.matmul(
                            gA[:], ones_sb[0:1, :], bias_sb[0:1, 0:512],
                            start=False, stop=False,
                        )
                        nc.tensor.matmul(
                            gB[:], ones_sb[0:1, :], bias_sb[0:1, 512:1024],
                            start=False, stop=False,
                        )
                    # recurrent GEMM for step t into row half r
                    rows = slice(r * 64, (r + 1) * 64)
                    tp = (0, r * 64)
                    for k in range(KT):
                        if t == 0:
                            hTprev = prev_hT[:, k, S - 1, :]
                        else:
                            hTprev = cur_hT[:, k, t - 1, :]
                        nc.tensor.matmul(
                            gA[rows, :], hTprev, whh_sb[:, k, 0:512],
                            start=False, stop=(k == KT - 1), tile_position=tp,
                        )
                        nc.tensor.matmul(
                            gB[rows, :], hTprev, whh_sb[:, k, 512:1024],
                            start=False, stop=(k == KT - 1), tile_position=tp,
                        )

                    # gates: [i f] = gA row half, [g o] = gB row half
                    nc.scalar.activation(s_sb[:, 0:512], gA[rows, :], AF.Sigmoid)
                    nc.scalar.activation(s_sb[:, 512:768], gB[rows, 0:256], AF.Tanh)
                    nc.scalar.activation(o_sb[:], gB[rows, 256:512], AF.Sigmoid)
                    # p = [i*g | f*c]  (c lives at s_sb[:, 768:1024])
                    nc.vector.tensor_mul(p_sb[:], s_sb[:, 0:512], s_sb[:, 512:1024])
                    nc.vector.tensor_add(
                        s_sb[:, 768:1024], p_sb[:, 0:256], p_sb[:, 256:512]
                    )
                    nc.scalar.activation(tc_sb[:], s_sb[:, 768:1024], AF.Tanh)
                    nc.vector.tensor_mul(h_sb[:, 0:128], o_sb[:, 0:128], tc_sb[:, 0:128])
                    nc.vector.tensor_mul(h_sb[:, 128:256], o_sb[:, 128:256], tc_sb[:, 128:256])
                    # transpose h to feature-major for next step / AG payload
                    for k in range(KT):
                        hp = pst.tile([128, 64], F32, tag="hp")
                        nc.tensor.transpose(
                            hp[:], h_sb[:, k * 128 : (k + 1) * 128], id_sb[:]
                        )
                        nc.vector.tensor_copy(cur_hT[:, k, t, :], hp[:])

                    if r == 1:
                        yp = psy.tile([128, V], F32, tag="yp")
                        for k in range(KT):
                            lhs = cur_hT[:, k, t - 1 : t + 1, :].rearrange(
                                "p s b -> p (s b)"
                            )
                            nc.tensor.matmul(
                                yp[:], lhs, fc_sb[:, k, :],
                                start=(k == 0), stop=(k == KT - 1),
                            )
                        nc.vector.tensor_copy(y_sb[:, t // 2, :], yp[:])

                # store y chunk: y_sb[(r b), pp, v] -> y[w, 2*pp+r, b, v]
                ydst = y_out[w].rearrange("(pp r) b v -> (r b) pp v", r=2)
                nc.sync.dma_start(ydst, y_sb[:])

    _split_multiwait(nc)
    nc.finalize()
    return nc


def _split_multiwait(nc):
    """Walrus in this toolchain accepts at most one sync-wait per instruction;
    split extras into single-wait NoOps inserted ahead in the same stream."""
    n = 0
    for bb in nc.main_func.blocks:
        new = []
        for ins in bb.instructions:
            si = ins.sync_info
            if si is not None and si.on_wait and len(si.on_wait) > 1:
                waits = list(si.on_wait)
                for w in waits[:-1]:
                    n += 1
                    new.append(mybir.InstNoOp(
                        name=f"wsplit-{n}", engine=ins.engine, ins=[], outs=[],
                        sync_info=mybir.SyncInfo(on_wait=[w], on_update=[]),
                    ))
                ins.sync_info = mybir.SyncInfo(
                    on_wait=[waits[-1]], on_update=list(si.on_update or [])
                )
            new.append(ins)
        bb.instructions = new
    return n


def _prep_inputs(x, emb, W_ih0, W_ih_rest, W_hh, b_ih, b_hh, fc_w, fc_b):
    x = np.asarray(x).astype(np.int64)
    emb = np.asarray(emb, dtype=np.float32)
    W_ih0 = np.asarray(W_ih0, dtype=np.float32)
    W_ih_rest = np.asarray(W_ih_rest, dtype=np.float32)
    W_hh = np.asarray(W_hh, dtype=np.float32)
    b_ih = np.asarray(b_ih, dtype=np.float32)
    b_hh = np.asarray(b_hh, dtype=np.float32)
    fc_w = np.asarray(fc_w, dtype=np.float32)

    # one-hot stream [C, 128, S*B]: oneh[c, tok, t*B + b] = 1 where tok = x[b, c*S+t]
    oneh = np.zeros((C, 128, S, B), dtype=np.float32)
    cs = (np.arange(T) // S)[None, :].repeat(B, 0)      # [B, T]
    ts_ = (np.arange(T) % S)[None, :].repeat(B, 0)
    bs = np.arange(B)[:, None].repeat(T, 1)
    oneh[cs.ravel(), x.ravel(), ts_.ravel(), bs.ravel()] = 1.0
    oneh = oneh.reshape(C, 128, S * B)

    M0 = np.zeros((H, G), dtype=np.float32)
    M0[:V] = emb @ W_ih0.T                               # [23, 1024]

    ident = np.eye(64, dtype=np.float32)
    zero_oneh = np.zeros_like(oneh)

    in_maps = []
    for l in range(NCORES):
        wih_l = M0 if l == 0 else W_ih_rest[l - 1].T     # [256, 1024] = Wih^T
        whh_l = W_hh[l].T                                # [256, 1024]
        bias_l = (b_ih[l] + b_hh[l]).astype(np.float32)  # [1024]
        bstream = np.tile(bias_l, (TICKS, 1))
        if l > 0:
            bstream[: LAG * l, 0:512] = FLUSH            # flush i,f before start
        in_maps.append(
            {
                "wih": np.ascontiguousarray(wih_l.reshape(KT, 128, G)),
                "whh": np.ascontiguousarray(whh_l.reshape(KT, 128, G)),
                "fcw": np.ascontiguousarray(fc_w.T.reshape(KT, 128, V)),
                "biasstream": bstream,
                "oneh": oneh if l == 0 else zero_oneh,
                "ident": ident,
                "gidx": (l * 128 + np.arange(128, dtype=np.int32)).reshape(128, 1),
            }
        )
    return in_maps


def kernel(x, emb, W_ih0, W_ih_rest, W_hh, b_ih, b_hh, fc_w, fc_b, _trace=False):
    if "nc" not in _cache:
        _cache["nc"] = _build_nc()
    nc = _cache["nc"]
    in_maps = _prep_inputs(x, emb, W_ih0, W_ih_rest, W_hh, b_ih, b_hh, fc_w, fc_b)
    res = run_bass_kernel_spmd(
        nc, in_maps, core_ids=list(range(NCORES)), trace=_trace
    )
    _cache["last_results"] = res
    y = res.results[NCORES - 1]["y"]                     # [TICKS, S, B, V]
    y = y[LAG * (L - 1) :]                               # [C, S, B, V]
    y = np.transpose(y.reshape(T, B, V), (1, 0, 2))      # [B, T, V]
    y = y + np.asarray(fc_b, dtype=np.float32)[None, None, :]
    return y.astype(np.float32)
